# revision 26
# baseline (speedup 1.0000x reference)
"""Trainium2 Bass kernel for AlignmentContrastiveLoss (8 NeuronCores, SPMD).

Reference semantics:
  im = im_set[:, 1:, :]           [256, 36, 1024]
  s  = s_seq[:, 1:-2, :]          [256, 32, 1024]
  align[i,j,n,m] = im[i,n] . s[j,m], zeroed where n >= im_len[i]-1 or m >= s_len[j]-3
  scores[i,j] = sum_m max_n align[i,j,n,m]
  loss = sum_i relu(M + max_{j!=i} scores[i,j] - scores[i,i])
       + sum_j relu(M + max_{i!=j} scores[i,j] - scores[j,j])

Sharding: data-parallel over images (32 per core), s replicated. Masking is
implemented by zeroing padded token rows before the matmul (dot with zero
vector == the reference's where(pad, 0, align)). Cross-core traffic is one
520-float AllGather of per-core column-max partials + scattered diagonals.
"""

import numpy as np

MARGIN = 0.2
B = 256          # global batch (images == sentences)
NCORES = 8
BI = B // NCORES  # images per core = 32
NREG = 36        # regions per image after stripping
NWORD = 32       # words per sentence after stripping
D = 1024
KT = D // 128    # 8 k-chunks
IM_TOK = BI * NREG      # 1152 = 9 * 128
S_TOK = B * NWORD       # 8192 = 64 * 128
IM_TILES = IM_TOK // 128  # 9
S_TILES = S_TOK // 128    # 64
BIG = 1.0e30
# region chunks for the main matmul: (token offset, ntok, nimg)
RCHUNKS = [(0, 432, 12), (432, 432, 12), (864, 288, 8)]
BLK = 520  # allgather block floats: 256 colmax | 256 diag-scatter | 1 cost_s | pad
USE_FP8 = True  # fp8e4 + DoubleRow for the big matmul (2 contraction elems/cell)


def fix_multiwaits(nc, mybir):
    """This toolchain's walrus accepts 1 wait per instruction (2 for
    EventSemaphore); Tile can emit more. Offload surplus waits onto
    inserted same-engine NoOps placed immediately before the instruction."""
    n_fix = 0
    for fn in nc.m.functions:
        for blk in fn.blocks:
            insts = blk.instructions
            i = 0
            while i < len(insts):
                inst = insts[i]
                si = inst.sync_info
                waits = list(si.on_wait) if si is not None and si.on_wait else []
                cap = 2 if isinstance(inst, mybir.InstEventSemaphore) else 1
                if len(waits) > cap:
                    surplus, keep = waits[:-cap], waits[-cap:]
                    si.on_wait = keep
                    for w in surplus:
                        nop = mybir.InstNoOp(
                            name=f"{inst.name}_wsplit{n_fix}",
                            engine=inst.engine,
                            ins=[],
                            outs=[],
                            sync_info=mybir.SyncInfo(on_wait=[w], on_update=[]),
                        )
                        insts.insert(i, nop)
                        n_fix += 1
                        i += 1
                i += 1
    return n_fix


DEFAULT_OPTS = {
    "alp_bufs": 3,
    "trp_bufs": 3,
    "stage_bufs": 4,
    "interleave_stage": True,
    "copy_split": 2,  # PSUM->SBUF copies per staged tile (1, 2, or 4)
    "cast_split": 1,  # masked-cast chunks per staged tile
}


def build_graph(opts=None):
    import concourse.bass as bass
    import concourse.mybir as mybir
    import concourse.tile as tile
    from concourse.masks import make_identity
    from contextlib import ExitStack

    opts = {**DEFAULT_OPTS, **(opts or {})}

    f32 = mybir.dt.float32
    bf16 = mybir.dt.bfloat16
    mmdt = mybir.dt.float8e4 if USE_FP8 else mybir.dt.bfloat16
    i32 = mybir.dt.int32
    perf_mode = mybir.MatmulPerfMode.DoubleRow if USE_FP8 else None
    kstep = 2 if USE_FP8 else 1
    ALU = mybir.AluOpType
    AX = mybir.AxisListType
    ACTF = mybir.ActivationFunctionType

    nc = bass.Bass()

    im_ext = nc.declare_dram_parameter("im_set", [BI, 37, D], f32, isOutput=False)
    s_ext = nc.declare_dram_parameter("s_seq", [B, 35, D], f32, isOutput=False)
    imlen_ext = nc.declare_dram_parameter("im_len", [BI], i32, isOutput=False)
    slen_ext = nc.declare_dram_parameter("s_len", [B], i32, isOutput=False)
    dmask_ext = nc.declare_dram_parameter("diag_mask", [B, BI], f32, isOutput=False)
    dmaskT_ext = nc.declare_dram_parameter("diag_maskT", [BI, B], f32, isOutput=False)
    out_ext = nc.declare_dram_parameter("out", [1], f32, isOutput=True)

    with tile.TileContext(nc) as tc, ExitStack() as top:
        # ---------------- constants ----------------
        const = top.enter_context(tc.tile_pool(name="const", bufs=1))
        ident_bf = const.tile([128, 128], bf16)
        make_identity(nc, ident_bf)
        ident_f32 = const.tile([128, 128], f32)
        make_identity(nc, ident_f32)
        ones32 = const.tile([32, 1], f32)
        nc.gpsimd.memset(ones32, 1.0)
        ones128 = const.tile([128, 1], f32)
        nc.gpsimd.memset(ones128, 1.0)

        # ---------------- token masks (device-side) ----------------
        # im: valid region n < im_len-1 ; s: valid word m < s_len-3
        mpool = top.enter_context(tc.tile_pool(name="masks", bufs=1))
        dram = top.enter_context(tc.tile_pool(name="dram", bufs=1, space="DRAM"))

        imlen_sb = mpool.tile([BI, 1], i32)
        nc.sync.dma_start(imlen_sb, imlen_ext.rearrange("(p o) -> p o", o=1))
        il_sb = mpool.tile([BI, 1], f32)
        nc.gpsimd.tensor_scalar(il_sb, imlen_sb, 1, None, op0=ALU.subtract)
        iota_r = mpool.tile([BI, NREG], f32)
        nc.gpsimd.iota(
            iota_r, pattern=[[1, NREG]], base=0, channel_multiplier=0,
            allow_small_or_imprecise_dtypes=True,
        )
        mask_im = mpool.tile([BI, NREG], f32)
        nc.gpsimd.tensor_scalar(mask_im, iota_r, il_sb, None, op0=ALU.is_lt)
        mask_im_dram = dram.tile([IM_TOK], f32)
        nc.sync.dma_start(mask_im_dram.rearrange("(p f) -> p f", f=NREG), mask_im)
        maskcol_im = mpool.tile([128, IM_TILES], f32)
        nc.sync.dma_start(maskcol_im, mask_im_dram.rearrange("(f p) -> p f", p=128))

        slen_sb = mpool.tile([128, 2], i32)
        nc.sync.dma_start(slen_sb, slen_ext.rearrange("(t p) -> p t", p=128))
        sl_sb = mpool.tile([128, 2], f32)
        nc.gpsimd.tensor_scalar(sl_sb, slen_sb, 3, None, op0=ALU.subtract)
        iota_w = mpool.tile([128, NWORD], f32)
        nc.gpsimd.iota(
            iota_w, pattern=[[1, NWORD]], base=0, channel_multiplier=0,
            allow_small_or_imprecise_dtypes=True,
        )
        mask_s = mpool.tile([128, 2, NWORD], f32)
        for t in range(2):
            nc.gpsimd.tensor_scalar(
                mask_s[:, t, :], iota_w, sl_sb[:, t:t + 1], None, op0=ALU.is_lt
            )
        mask_s_dram = dram.tile([S_TOK], f32)
        for t in range(2):
            nc.sync.dma_start(
                mask_s_dram[t * 4096:(t + 1) * 4096].rearrange("(p f) -> p f", f=NWORD),
                mask_s[:, t, :],
            )
        maskcol_s = mpool.tile([128, S_TILES], f32)
        nc.sync.dma_start(maskcol_s, mask_s_dram.rearrange("(f p) -> p f", p=128))

        # diag masks (sharding metadata inputs)
        dmask_sb = mpool.tile([128, 2, BI], f32)
        nc.sync.dma_start(dmask_sb, dmask_ext.rearrange("(t p) i -> p t i", p=128))
        dmaskT_sb = mpool.tile([BI, 2, 128], f32)
        nc.sync.dma_start(dmaskT_sb, dmaskT_ext.rearrange("p (t f) -> p t f", f=128))

        # ---------------- persistent big buffers ----------------
        big = top.enter_context(tc.tile_pool(name="big", bufs=1))
        imT = big.tile([128, KT, IM_TOK], mmdt)   # [d%128, d//128, im token]
        sT = big.tile([128, KT, S_TOK], mmdt)     # [d%128, d//128, s token]
        maxima = big.tile([128, S_TILES, BI], bf16)  # per (word, wtile, img) region-max
        scores_sb = big.tile([128, 2, BI], f32)     # [sent%128, sent//128, img]
        scoresT_sb = big.tile([BI, S_TILES, 4], f32)  # [img, wtile, sent%4] == [img, sent]

        with ExitStack() as mid:
            stage = mid.enter_context(
                tc.tile_pool(name="stage", bufs=opts["stage_bufs"])
            )
            trp = mid.enter_context(
                tc.tile_pool(name="trp", bufs=opts["trp_bufs"], space="PSUM")
            )
            alp = mid.enter_context(
                tc.tile_pool(name="alp", bufs=opts["alp_bufs"], space="PSUM")
            )
            mxp = mid.enter_context(tc.tile_pool(name="mxp", bufs=1, space="PSUM"))
            scp = mid.enter_context(tc.tile_pool(name="scp", bufs=1, space="PSUM"))

            def stage_tokens(tiles, dst, maskcol, dma_fn, tagpfx):
                # software-pipelined: tile t+1's DMA+cast is emitted BEFORE
                # tile t's copies so the cast never queues behind them on
                # the Scalar engine (the PE transposes wait on the cast).
                tiles = list(tiles)
                tbfs = {}

                def load(t):
                    tf32 = stage.tile([128, D], f32, tag=f"{tagpfx}f32")
                    dma_fn(t, tf32)
                    tbf = stage.tile([128, D], bf16, tag=f"{tagpfx}bf")
                    nc.scalar.activation(
                        tbf, tf32, ACTF.Copy, scale=maskcol[:, t:t + 1]
                    )
                    tbfs[t] = tbf

                load(tiles[0])
                for i, t in enumerate(tiles):
                    if i + 1 < len(tiles):
                        load(tiles[i + 1])
                    tbf = tbfs.pop(t)
                    ng = KT // opts["copy_split"]
                    for kg in range(opts["copy_split"]):
                        ps = trp.tile([128, ng, 128], bf16, tag="trps")
                        for kk in range(ng):
                            k = ng * kg + kk
                            nc.tensor.transpose(
                                ps[:, kk, :], tbf[:, 128 * k:128 * (k + 1)],
                                ident_bf,
                            )
                        nc.scalar.copy(
                            dst[:, ng * kg:ng * (kg + 1), 128 * t:128 * (t + 1)],
                            ps,
                        )

            def im_dma(t, tf32):
                # tile t covers flat im tokens [128t, 128t+128); token = 36*i + n
                t0 = 128 * t
                i_lo, i_hi = t0 // NREG, (t0 + 127) // NREG
                for i in range(i_lo, i_hi + 1):
                    a = max(0, NREG * i - t0)
                    b = min(128, NREG * (i + 1) - t0)
                    n0 = t0 + a - NREG * i
                    nc.sync.dma_start(
                        tf32[a:b, :], im_ext[i, 1 + n0:1 + n0 + (b - a), :]
                    )

            def s_dma(t, tf32):
                # tile t covers 4 sentences' words: s_seq[4t:4t+4, 1:33, :]
                for j in range(4):
                    nc.sync.dma_start(
                        tf32[NWORD * j:NWORD * (j + 1), :],
                        s_ext[4 * t + j, 1:1 + NWORD, :],
                    )

            # rc chunk 0 consumes im tiles 0-3 and s tiles in order; stage
            # just enough im first so the matmul sweep starts early
            if opts["interleave_stage"]:
                stage_tokens(range(4), imT, maskcol_im, im_dma, "im")
                stage_tokens(range(8), sT, maskcol_s, s_dma, "s")
                stage_tokens(range(4, IM_TILES), imT, maskcol_im, im_dma, "im")
                stage_tokens(range(8, S_TILES), sT, maskcol_s, s_dma, "s")
            else:
                stage_tokens(range(IM_TILES), imT, maskcol_im, im_dma, "im")
                stage_tokens(range(S_TILES), sT, maskcol_s, s_dma, "s")

            # ---------------- main matmul + region-max + word-sum ----------------
            for rci, (toff, ntok, nimg) in enumerate(RCHUNKS):
                ib = toff // NREG
                for wt in range(S_TILES):
                    pal = alp.tile([128, nimg, NREG], f32, tag="align")
                    for k in range(0, KT, kstep):
                        lhsT = (
                            sT[:, k:k + 2, 128 * wt:128 * (wt + 1)]
                            if USE_FP8 else sT[:, k, 128 * wt:128 * (wt + 1)]
                        )
                        rhs = (
                            imT[:, k:k + 2, toff:toff + ntok]
                            if USE_FP8 else imT[:, k, toff:toff + ntok]
                        )
                        nc.tensor.matmul(
                            pal.rearrange("p a b -> p (a b)"),
                            lhsT=lhsT,
                            rhs=rhs,
                            start=(k == 0),
                            stop=(k + kstep >= KT),
                            perf_mode=perf_mode,
                        )
                    nc.vector.tensor_reduce(
                        maxima[:, wt, ib:ib + nimg], pal, axis=AX.X, op=ALU.max
                    )
                    if rci == 2 and wt % 4 == 3:
                        # word-sum for group g: imgs to partitions, sum words
                        g = wt // 4
                        mx_ps = mxp.tile([BI, 4, 128], bf16, tag="mxps", name=f"mx{g}")
                        for u in range(4):
                            nc.tensor.transpose(
                                mx_ps[:, u, :], maxima[:, 4 * g + u, :], ident_bf
                            )
                        nc.vector.tensor_reduce(
                            scoresT_sb[:, 4 * g:4 * (g + 1), :],
                            mx_ps.rearrange("p u (v w) -> p u v w", w=NWORD),
                            axis=AX.X,
                            op=ALU.add,
                        )
            # transpose back: sentences to partitions
            sc_ps = scp.tile([128, 2, BI], f32)
            for t in range(2):
                nc.tensor.transpose(
                    sc_ps[:, t, :],
                    scoresT_sb[:, 32 * t:32 * (t + 1), :].rearrange(
                        "p a b -> p (a b)"
                    ),
                    ident_f32[:BI, :BI],
                )
                nc.vector.tensor_copy(scores_sb[:, t, :], sc_ps[:, t, :])

        # ---------------- loss tail ----------------
        with ExitStack() as tail:
            tp = tail.enter_context(tc.tile_pool(name="tailp", bufs=1, space="PSUM"))
            ts = tail.enter_context(tc.tile_pool(name="tails", bufs=1))

            # col-max over local images (diag excluded) + scattered diag
            masked = ts.tile([128, 2, BI], f32)
            nc.vector.scalar_tensor_tensor(
                masked, dmask_sb, -BIG, scores_sb, op0=ALU.mult, op1=ALU.add
            )
            colmax_p = ts.tile([128, 2], f32)
            nc.vector.tensor_reduce(colmax_p, masked, axis=AX.X, op=ALU.max)
            dtmp = ts.tile([128, 2, BI], f32)
            nc.vector.tensor_mul(dtmp, dmask_sb, scores_sb)
            dscat = ts.tile([128, 2], f32)
            nc.vector.tensor_reduce(dscat, dtmp, axis=AX.X, op=ALU.add)

            # row-max over sentences (diag excluded); scoresT_sb is [img, sent]
            scT_flat = scoresT_sb.rearrange("p a b -> p (a b)")
            dmaskT_flat = dmaskT_sb.rearrange("p a b -> p (a b)")
            maskedT = ts.tile([BI, B], f32)
            nc.vector.scalar_tensor_tensor(
                maskedT, dmaskT_flat, -BIG, scT_flat, op0=ALU.mult, op1=ALU.add
            )
            rowmax = ts.tile([BI, 1], f32)
            nc.vector.tensor_reduce(rowmax, maskedT, axis=AX.X, op=ALU.max)
            dT_tmp = ts.tile([BI, B], f32)
            nc.vector.tensor_mul(dT_tmp, dmaskT_flat, scT_flat)
            d_row = ts.tile([BI, 1], f32)
            nc.vector.tensor_reduce(d_row, dT_tmp, axis=AX.X, op=ALU.add)

            cost_s = ts.tile([BI, 1], f32)
            nc.vector.tensor_sub(cost_s, rowmax, d_row)
            nc.vector.tensor_scalar(
                cost_s, cost_s, MARGIN, 0.0, op0=ALU.add, op1=ALU.max
            )
            cs_ps = tp.tile([1, 1], f32)
            nc.tensor.matmul(cs_ps, lhsT=ones32, rhs=cost_s, start=True, stop=True)
            cs_sb = ts.tile([1, 8], f32)
            nc.gpsimd.memset(cs_sb, 0.0)
            nc.vector.tensor_copy(cs_sb[:, 0:1], cs_ps)

            # pack allgather block: [0:256) colmax | [256:512) dscat | 512 cost_s
            blk = dram.tile([BLK], f32)
            nc.sync.dma_start(
                blk[0:256].rearrange("(t p) -> p t", p=128), colmax_p
            )
            nc.sync.dma_start(
                blk[256:512].rearrange("(t p) -> p t", p=128), dscat
            )
            nc.sync.dma_start(blk[512:520], cs_sb[0, :])
            gath = dram.tile([NCORES, BLK], f32, addr_space="Shared")
            nc.gpsimd.collective_compute(
                "AllGather",
                ALU.bypass,
                ins=[blk.opt()],
                outs=[gath.opt()],
                replica_groups=[list(range(NCORES))],
            )

            # redundant final reduction on every core
            g_cm = ts.tile([128, 2, NCORES], f32)
            g_d = ts.tile([128, 2, NCORES], f32)
            for t in range(2):
                nc.sync.dma_start(
                    g_cm[:, t, :],
                    gath[:, 128 * t:128 * (t + 1)].rearrange("c p -> p c"),
                )
                nc.sync.dma_start(
                    g_d[:, t, :],
                    gath[:, 256 + 128 * t:256 + 128 * (t + 1)].rearrange("c p -> p c"),
                )
            g_cs = ts.tile([1, NCORES], f32)
            nc.sync.dma_start(g_cs, gath[:, 512:513].rearrange("a b -> b a"))

            colmax_g = ts.tile([128, 2], f32)
            nc.vector.tensor_reduce(colmax_g, g_cm, axis=AX.X, op=ALU.max)
            d_all = ts.tile([128, 2], f32)
            nc.vector.tensor_reduce(d_all, g_d, axis=AX.X, op=ALU.add)
            cim = ts.tile([128, 2], f32)
            nc.vector.tensor_sub(cim, colmax_g, d_all)
            nc.vector.tensor_scalar(cim, cim, MARGIN, 0.0, op0=ALU.add, op1=ALU.max)
            cim_r = ts.tile([128, 1], f32)
            nc.vector.tensor_reduce(cim_r, cim, axis=AX.X, op=ALU.add)
            tot_ps = tp.tile([1, 1], f32)
            nc.tensor.matmul(tot_ps, lhsT=ones128, rhs=cim_r, start=True, stop=True)
            cs_tot = ts.tile([1, 1], f32)
            nc.vector.tensor_reduce(cs_tot, g_cs, axis=AX.X, op=ALU.add)
            total = ts.tile([1, 1], f32)
            nc.vector.tensor_add(total, tot_ps, cs_tot)
            nc.sync.dma_start(out_ext[0:1], total[0, :])

    fix_multiwaits(nc, mybir)
    return nc


_CACHE = {}


def _get_nc():
    if "nc" not in _CACHE:
        _CACHE["nc"] = build_graph()
    return _CACHE["nc"]


def make_in_maps(im_set, s_seq, im_len, s_len):
    im_set = np.ascontiguousarray(im_set, dtype=np.float32)
    s_seq = np.ascontiguousarray(s_seq, dtype=np.float32)
    im_len = np.ascontiguousarray(im_len, dtype=np.int32)
    s_len = np.ascontiguousarray(s_len, dtype=np.int32)
    in_maps = []
    for c in range(NCORES):
        dm = np.zeros((B, BI), dtype=np.float32)
        for i in range(BI):
            dm[BI * c + i, i] = 1.0
        in_maps.append({
            "im_set": im_set[BI * c:BI * (c + 1)],
            "s_seq": s_seq,
            "im_len": im_len[BI * c:BI * (c + 1)],
            "s_len": s_len,
            "diag_mask": dm,
            "diag_maskT": np.ascontiguousarray(dm.T),
        })
    return in_maps


def kernel(im_set, s_seq, im_len, s_len):
    import time
    from concourse.bass_utils import run_bass_kernel_spmd

    nc = _get_nc()
    in_maps = make_in_maps(im_set, s_seq, im_len, s_len)
    last = None
    for attempt in range(3):
        try:
            res = run_bass_kernel_spmd(nc, in_maps, core_ids=list(range(NCORES)))
            return np.asarray(
                res.results[0]["out"], dtype=np.float32
            ).reshape(())[()]
        except Exception as e:  # transient device-unrecoverable happens
            last = e
            time.sleep(30 * (attempt + 1))
    raise last



# revision 27
# speedup vs baseline: 1.0256x; 1.0256x over previous
"""Trainium2 Bass kernel for AlignmentContrastiveLoss (8 NeuronCores, SPMD).

Reference semantics:
  im = im_set[:, 1:, :]           [256, 36, 1024]
  s  = s_seq[:, 1:-2, :]          [256, 32, 1024]
  align[i,j,n,m] = im[i,n] . s[j,m], zeroed where n >= im_len[i]-1 or m >= s_len[j]-3
  scores[i,j] = sum_m max_n align[i,j,n,m]
  loss = sum_i relu(M + max_{j!=i} scores[i,j] - scores[i,i])
       + sum_j relu(M + max_{i!=j} scores[i,j] - scores[j,j])

Sharding: data-parallel over images (32 per core), s replicated. Masking is
implemented by zeroing padded token rows before the matmul (dot with zero
vector == the reference's where(pad, 0, align)). Cross-core traffic is one
520-float AllGather of per-core column-max partials + scattered diagonals.
"""

import numpy as np

MARGIN = 0.2
B = 256          # global batch (images == sentences)
NCORES = 8
BI = B // NCORES  # images per core = 32
NREG = 36        # regions per image after stripping
NWORD = 32       # words per sentence after stripping
D = 1024
KT = D // 128    # 8 k-chunks
IM_TOK = BI * NREG      # 1152 = 9 * 128
S_TOK = B * NWORD       # 8192 = 64 * 128
IM_TILES = IM_TOK // 128  # 9
S_TILES = S_TOK // 128    # 64
BIG = 1.0e30
# region chunks for the main matmul: (token offset, ntok, nimg)
RCHUNKS = [(0, 432, 12), (432, 432, 12), (864, 288, 8)]
BLK = 520  # allgather block floats: 256 colmax | 256 diag-scatter | 1 cost_s | pad
USE_FP8 = True  # fp8e4 + DoubleRow for the big matmul (2 contraction elems/cell)


def fix_multiwaits(nc, mybir):
    """This toolchain's walrus accepts 1 wait per instruction (2 for
    EventSemaphore); Tile can emit more. Offload surplus waits onto
    inserted same-engine NoOps placed immediately before the instruction."""
    n_fix = 0
    for fn in nc.m.functions:
        for blk in fn.blocks:
            insts = blk.instructions
            i = 0
            while i < len(insts):
                inst = insts[i]
                si = inst.sync_info
                waits = list(si.on_wait) if si is not None and si.on_wait else []
                cap = 2 if isinstance(inst, mybir.InstEventSemaphore) else 1
                if len(waits) > cap:
                    surplus, keep = waits[:-cap], waits[-cap:]
                    si.on_wait = keep
                    for w in surplus:
                        nop = mybir.InstNoOp(
                            name=f"{inst.name}_wsplit{n_fix}",
                            engine=inst.engine,
                            ins=[],
                            outs=[],
                            sync_info=mybir.SyncInfo(on_wait=[w], on_update=[]),
                        )
                        insts.insert(i, nop)
                        n_fix += 1
                        i += 1
                i += 1
    return n_fix


DEFAULT_OPTS = {
    "alp_bufs": 3,
    "trp_bufs": 3,
    "stage_bufs": 3,
    "interleave_stage": True,
    "copy_split": 2,  # PSUM->SBUF copies per staged tile (1, 2, or 4)
    "cast_split": 1,  # masked-cast chunks per staged tile
}


def build_graph(opts=None):
    import concourse.bass as bass
    import concourse.mybir as mybir
    import concourse.tile as tile
    from concourse.masks import make_identity
    from contextlib import ExitStack

    opts = {**DEFAULT_OPTS, **(opts or {})}

    f32 = mybir.dt.float32
    bf16 = mybir.dt.bfloat16
    mmdt = mybir.dt.float8e4 if USE_FP8 else mybir.dt.bfloat16
    i32 = mybir.dt.int32
    perf_mode = mybir.MatmulPerfMode.DoubleRow if USE_FP8 else None
    kstep = 2 if USE_FP8 else 1
    ALU = mybir.AluOpType
    AX = mybir.AxisListType
    ACTF = mybir.ActivationFunctionType

    nc = bass.Bass()

    im_ext = nc.declare_dram_parameter("im_set", [BI, 37, D], f32, isOutput=False)
    s_ext = nc.declare_dram_parameter("s_seq", [B, 35, D], f32, isOutput=False)
    imlen_ext = nc.declare_dram_parameter("im_len", [BI], i32, isOutput=False)
    slen_ext = nc.declare_dram_parameter("s_len", [B], i32, isOutput=False)
    dmask_ext = nc.declare_dram_parameter("diag_mask", [B, BI], f32, isOutput=False)
    dmaskT_ext = nc.declare_dram_parameter("diag_maskT", [BI, B], f32, isOutput=False)
    out_ext = nc.declare_dram_parameter("out", [1], f32, isOutput=True)

    with tile.TileContext(nc) as tc, ExitStack() as top:
        # ---------------- constants ----------------
        const = top.enter_context(tc.tile_pool(name="const", bufs=1))
        ident_bf = const.tile([128, 128], bf16)
        make_identity(nc, ident_bf)
        ident_f32 = const.tile([128, 128], f32)
        make_identity(nc, ident_f32)
        ones32 = const.tile([32, 1], f32)
        nc.gpsimd.memset(ones32, 1.0)
        ones128 = const.tile([128, 1], f32)
        nc.gpsimd.memset(ones128, 1.0)

        # ---------------- token masks (device-side) ----------------
        # im: valid region n < im_len-1 ; s: valid word m < s_len-3
        mpool = top.enter_context(tc.tile_pool(name="masks", bufs=1))
        dram = top.enter_context(tc.tile_pool(name="dram", bufs=1, space="DRAM"))

        imlen_sb = mpool.tile([BI, 1], i32)
        nc.sync.dma_start(imlen_sb, imlen_ext.rearrange("(p o) -> p o", o=1))
        il_sb = mpool.tile([BI, 1], f32)
        nc.gpsimd.tensor_scalar(il_sb, imlen_sb, 1, None, op0=ALU.subtract)
        iota_r = mpool.tile([BI, NREG], f32)
        nc.gpsimd.iota(
            iota_r, pattern=[[1, NREG]], base=0, channel_multiplier=0,
            allow_small_or_imprecise_dtypes=True,
        )
        mask_im = mpool.tile([BI, NREG], f32)
        nc.gpsimd.tensor_scalar(mask_im, iota_r, il_sb, None, op0=ALU.is_lt)
        mask_im_dram = dram.tile([IM_TOK], f32)
        nc.sync.dma_start(mask_im_dram.rearrange("(p f) -> p f", f=NREG), mask_im)
        maskcol_im = mpool.tile([128, IM_TILES], f32)
        nc.sync.dma_start(maskcol_im, mask_im_dram.rearrange("(f p) -> p f", p=128))

        slen_sb = mpool.tile([128, 2], i32)
        nc.sync.dma_start(slen_sb, slen_ext.rearrange("(t p) -> p t", p=128))
        sl_sb = mpool.tile([128, 2], f32)
        nc.gpsimd.tensor_scalar(sl_sb, slen_sb, 3, None, op0=ALU.subtract)
        iota_w = mpool.tile([128, NWORD], f32)
        nc.gpsimd.iota(
            iota_w, pattern=[[1, NWORD]], base=0, channel_multiplier=0,
            allow_small_or_imprecise_dtypes=True,
        )
        mask_s = mpool.tile([128, 2, NWORD], f32)
        for t in range(2):
            nc.gpsimd.tensor_scalar(
                mask_s[:, t, :], iota_w, sl_sb[:, t:t + 1], None, op0=ALU.is_lt
            )
        mask_s_dram = dram.tile([S_TOK], f32)
        for t in range(2):
            nc.sync.dma_start(
                mask_s_dram[t * 4096:(t + 1) * 4096].rearrange("(p f) -> p f", f=NWORD),
                mask_s[:, t, :],
            )
        maskcol_s = mpool.tile([128, S_TILES], f32)
        nc.sync.dma_start(maskcol_s, mask_s_dram.rearrange("(f p) -> p f", p=128))

        # diag masks (sharding metadata inputs)
        dmask_sb = mpool.tile([128, 2, BI], f32)
        nc.sync.dma_start(dmask_sb, dmask_ext.rearrange("(t p) i -> p t i", p=128))
        dmaskT_sb = mpool.tile([BI, 2, 128], f32)
        nc.sync.dma_start(dmaskT_sb, dmaskT_ext.rearrange("p (t f) -> p t f", f=128))

        # ---------------- persistent big buffers ----------------
        big = top.enter_context(tc.tile_pool(name="big", bufs=1))
        imT = big.tile([128, KT, IM_TOK], mmdt)   # [d%128, d//128, im token]
        sT = big.tile([128, KT, S_TOK], mmdt)     # [d%128, d//128, s token]
        maxima = big.tile([128, S_TILES, BI], bf16)  # per (word, wtile, img) region-max
        scores_sb = big.tile([128, 2, BI], f32)     # [sent%128, sent//128, img]
        scoresT_sb = big.tile([BI, S_TILES, 4], f32)  # [img, wtile, sent%4] == [img, sent]

        with ExitStack() as mid:
            stage = mid.enter_context(
                tc.tile_pool(name="stage", bufs=opts["stage_bufs"])
            )
            trp = mid.enter_context(
                tc.tile_pool(name="trp", bufs=opts["trp_bufs"], space="PSUM")
            )
            alp = mid.enter_context(
                tc.tile_pool(name="alp", bufs=opts["alp_bufs"], space="PSUM")
            )
            mxp = mid.enter_context(tc.tile_pool(name="mxp", bufs=1, space="PSUM"))
            scp = mid.enter_context(tc.tile_pool(name="scp", bufs=1, space="PSUM"))

            def stage_tokens(tiles, dst, maskcol, dma_fn, tagpfx):
                for t in tiles:
                    tf32 = stage.tile([128, D], f32, tag=f"{tagpfx}f32")
                    dma_fn(t, tf32)
                    tbf = stage.tile([128, D], bf16, tag=f"{tagpfx}bf")
                    nd = D // opts["cast_split"]
                    for cg in range(opts["cast_split"]):
                        nc.scalar.activation(
                            tbf[:, nd * cg:nd * (cg + 1)],
                            tf32[:, nd * cg:nd * (cg + 1)],
                            ACTF.Copy, scale=maskcol[:, t:t + 1],
                        )
                    ng = KT // opts["copy_split"]
                    for kg in range(opts["copy_split"]):
                        ps = trp.tile([128, ng, 128], bf16, tag="trps")
                        for kk in range(ng):
                            k = ng * kg + kk
                            nc.tensor.transpose(
                                ps[:, kk, :], tbf[:, 128 * k:128 * (k + 1)],
                                ident_bf,
                            )
                        nc.scalar.copy(
                            dst[:, ng * kg:ng * (kg + 1), 128 * t:128 * (t + 1)],
                            ps,
                        )

            def im_dma(t, tf32):
                # tile t covers flat im tokens [128t, 128t+128); token = 36*i + n
                t0 = 128 * t
                i_lo, i_hi = t0 // NREG, (t0 + 127) // NREG
                for i in range(i_lo, i_hi + 1):
                    a = max(0, NREG * i - t0)
                    b = min(128, NREG * (i + 1) - t0)
                    n0 = t0 + a - NREG * i
                    nc.sync.dma_start(
                        tf32[a:b, :], im_ext[i, 1 + n0:1 + n0 + (b - a), :]
                    )

            def s_dma(t, tf32):
                # tile t covers 4 sentences' words: s_seq[4t:4t+4, 1:33, :]
                for j in range(4):
                    nc.sync.dma_start(
                        tf32[NWORD * j:NWORD * (j + 1), :],
                        s_ext[4 * t + j, 1:1 + NWORD, :],
                    )

            # rc chunk 0 consumes im tiles 0-3 and s tiles in order; stage
            # just enough im first so the matmul sweep starts early
            if opts["interleave_stage"]:
                stage_tokens(range(4), imT, maskcol_im, im_dma, "im")
                stage_tokens(range(8), sT, maskcol_s, s_dma, "s")
                stage_tokens(range(4, IM_TILES), imT, maskcol_im, im_dma, "im")
                stage_tokens(range(8, S_TILES), sT, maskcol_s, s_dma, "s")
            else:
                stage_tokens(range(IM_TILES), imT, maskcol_im, im_dma, "im")
                stage_tokens(range(S_TILES), sT, maskcol_s, s_dma, "s")

            # ---------------- main matmul + region-max + word-sum ----------------
            for rci, (toff, ntok, nimg) in enumerate(RCHUNKS):
                ib = toff // NREG
                for wt in range(S_TILES):
                    pal = alp.tile([128, nimg, NREG], f32, tag="align")
                    for k in range(0, KT, kstep):
                        lhsT = (
                            sT[:, k:k + 2, 128 * wt:128 * (wt + 1)]
                            if USE_FP8 else sT[:, k, 128 * wt:128 * (wt + 1)]
                        )
                        rhs = (
                            imT[:, k:k + 2, toff:toff + ntok]
                            if USE_FP8 else imT[:, k, toff:toff + ntok]
                        )
                        nc.tensor.matmul(
                            pal.rearrange("p a b -> p (a b)"),
                            lhsT=lhsT,
                            rhs=rhs,
                            start=(k == 0),
                            stop=(k + kstep >= KT),
                            perf_mode=perf_mode,
                        )
                    nc.vector.tensor_reduce(
                        maxima[:, wt, ib:ib + nimg], pal, axis=AX.X, op=ALU.max
                    )
                    if rci == 2 and wt % 4 == 3:
                        # word-sum for group g: imgs to partitions, sum words
                        g = wt // 4
                        mx_ps = mxp.tile([BI, 4, 128], bf16, tag="mxps", name=f"mx{g}")
                        for u in range(4):
                            nc.tensor.transpose(
                                mx_ps[:, u, :], maxima[:, 4 * g + u, :], ident_bf
                            )
                        nc.vector.tensor_reduce(
                            scoresT_sb[:, 4 * g:4 * (g + 1), :],
                            mx_ps.rearrange("p u (v w) -> p u v w", w=NWORD),
                            axis=AX.X,
                            op=ALU.add,
                        )
            # transpose back: sentences to partitions
            sc_ps = scp.tile([128, 2, BI], f32)
            for t in range(2):
                nc.tensor.transpose(
                    sc_ps[:, t, :],
                    scoresT_sb[:, 32 * t:32 * (t + 1), :].rearrange(
                        "p a b -> p (a b)"
                    ),
                    ident_f32[:BI, :BI],
                )
                nc.vector.tensor_copy(scores_sb[:, t, :], sc_ps[:, t, :])

        # ---------------- loss tail ----------------
        with ExitStack() as tail:
            tp = tail.enter_context(tc.tile_pool(name="tailp", bufs=1, space="PSUM"))
            ts = tail.enter_context(tc.tile_pool(name="tails", bufs=1))

            # col-max over local images (diag excluded) + scattered diag
            masked = ts.tile([128, 2, BI], f32)
            nc.vector.scalar_tensor_tensor(
                masked, dmask_sb, -BIG, scores_sb, op0=ALU.mult, op1=ALU.add
            )
            colmax_p = ts.tile([128, 2], f32)
            nc.vector.tensor_reduce(colmax_p, masked, axis=AX.X, op=ALU.max)
            dtmp = ts.tile([128, 2, BI], f32)
            nc.vector.tensor_mul(dtmp, dmask_sb, scores_sb)
            dscat = ts.tile([128, 2], f32)
            nc.vector.tensor_reduce(dscat, dtmp, axis=AX.X, op=ALU.add)

            # row-max over sentences (diag excluded); scoresT_sb is [img, sent]
            scT_flat = scoresT_sb.rearrange("p a b -> p (a b)")
            dmaskT_flat = dmaskT_sb.rearrange("p a b -> p (a b)")
            maskedT = ts.tile([BI, B], f32)
            nc.vector.scalar_tensor_tensor(
                maskedT, dmaskT_flat, -BIG, scT_flat, op0=ALU.mult, op1=ALU.add
            )
            rowmax = ts.tile([BI, 1], f32)
            nc.vector.tensor_reduce(rowmax, maskedT, axis=AX.X, op=ALU.max)
            dT_tmp = ts.tile([BI, B], f32)
            nc.vector.tensor_mul(dT_tmp, dmaskT_flat, scT_flat)
            d_row = ts.tile([BI, 1], f32)
            nc.vector.tensor_reduce(d_row, dT_tmp, axis=AX.X, op=ALU.add)

            cost_s = ts.tile([BI, 1], f32)
            nc.vector.tensor_sub(cost_s, rowmax, d_row)
            nc.vector.tensor_scalar(
                cost_s, cost_s, MARGIN, 0.0, op0=ALU.add, op1=ALU.max
            )
            cs_ps = tp.tile([1, 1], f32)
            nc.tensor.matmul(cs_ps, lhsT=ones32, rhs=cost_s, start=True, stop=True)
            cs_sb = ts.tile([1, 8], f32)
            nc.gpsimd.memset(cs_sb, 0.0)
            nc.vector.tensor_copy(cs_sb[:, 0:1], cs_ps)

            # pack allgather block: [0:256) colmax | [256:512) dscat | 512 cost_s
            blk = dram.tile([BLK], f32)
            nc.sync.dma_start(
                blk[0:256].rearrange("(t p) -> p t", p=128), colmax_p
            )
            nc.sync.dma_start(
                blk[256:512].rearrange("(t p) -> p t", p=128), dscat
            )
            nc.sync.dma_start(blk[512:520], cs_sb[0, :])
            gath = dram.tile([NCORES, BLK], f32, addr_space="Shared")
            nc.gpsimd.collective_compute(
                "AllGather",
                ALU.bypass,
                ins=[blk.opt()],
                outs=[gath.opt()],
                replica_groups=[list(range(NCORES))],
            )

            # redundant final reduction on every core
            g_cm = ts.tile([128, 2, NCORES], f32)
            g_d = ts.tile([128, 2, NCORES], f32)
            for t in range(2):
                nc.sync.dma_start(
                    g_cm[:, t, :],
                    gath[:, 128 * t:128 * (t + 1)].rearrange("c p -> p c"),
                )
                nc.sync.dma_start(
                    g_d[:, t, :],
                    gath[:, 256 + 128 * t:256 + 128 * (t + 1)].rearrange("c p -> p c"),
                )
            g_cs = ts.tile([1, NCORES], f32)
            nc.sync.dma_start(g_cs, gath[:, 512:513].rearrange("a b -> b a"))

            colmax_g = ts.tile([128, 2], f32)
            nc.vector.tensor_reduce(colmax_g, g_cm, axis=AX.X, op=ALU.max)
            d_all = ts.tile([128, 2], f32)
            nc.vector.tensor_reduce(d_all, g_d, axis=AX.X, op=ALU.add)
            cim = ts.tile([128, 2], f32)
            nc.vector.tensor_sub(cim, colmax_g, d_all)
            nc.vector.tensor_scalar(cim, cim, MARGIN, 0.0, op0=ALU.add, op1=ALU.max)
            cim_r = ts.tile([128, 1], f32)
            nc.vector.tensor_reduce(cim_r, cim, axis=AX.X, op=ALU.add)
            tot_ps = tp.tile([1, 1], f32)
            nc.tensor.matmul(tot_ps, lhsT=ones128, rhs=cim_r, start=True, stop=True)
            cs_tot = ts.tile([1, 1], f32)
            nc.vector.tensor_reduce(cs_tot, g_cs, axis=AX.X, op=ALU.add)
            total = ts.tile([1, 1], f32)
            nc.vector.tensor_add(total, tot_ps, cs_tot)
            nc.sync.dma_start(out_ext[0:1], total[0, :])

    fix_multiwaits(nc, mybir)
    return nc


_CACHE = {}


def _get_nc():
    if "nc" not in _CACHE:
        _CACHE["nc"] = build_graph()
    return _CACHE["nc"]


def make_in_maps(im_set, s_seq, im_len, s_len):
    im_set = np.ascontiguousarray(im_set, dtype=np.float32)
    s_seq = np.ascontiguousarray(s_seq, dtype=np.float32)
    im_len = np.ascontiguousarray(im_len, dtype=np.int32)
    s_len = np.ascontiguousarray(s_len, dtype=np.int32)
    in_maps = []
    for c in range(NCORES):
        dm = np.zeros((B, BI), dtype=np.float32)
        for i in range(BI):
            dm[BI * c + i, i] = 1.0
        in_maps.append({
            "im_set": im_set[BI * c:BI * (c + 1)],
            "s_seq": s_seq,
            "im_len": im_len[BI * c:BI * (c + 1)],
            "s_len": s_len,
            "diag_mask": dm,
            "diag_maskT": np.ascontiguousarray(dm.T),
        })
    return in_maps


def kernel(im_set, s_seq, im_len, s_len):
    import time
    from concourse.bass_utils import run_bass_kernel_spmd

    nc = _get_nc()
    in_maps = make_in_maps(im_set, s_seq, im_len, s_len)
    last = None
    for attempt in range(3):
        try:
            res = run_bass_kernel_spmd(nc, in_maps, core_ids=list(range(NCORES)))
            return np.asarray(
                res.results[0]["out"], dtype=np.float32
            ).reshape(())[()]
        except Exception as e:  # transient device-unrecoverable happens
            last = e
            time.sleep(30 * (attempt + 1))
    raise last

